# revision 11
# baseline (speedup 1.0000x reference)
"""BatchAll triplet loss (multi-module variant) on 8 Trainium2 NeuronCores.

Math: labels = [0..191, 0..191] -- each anchor i has exactly ONE valid positive
j = (i+192) % 384, so the (i,j,k) cubic triplet tensor collapses to (i,k):

    loss_terms[i,k] = relu(d(i, p(i)) - d(i,k) + margin) * pm[i,k] * valid[i,k]
    out = sum(loss_terms) / (count(loss_terms > EPS) + EPS)

d(i,k) = sqrt(relu(2 + delta - 2*G[i,k]*rn_i*rn_k)) with raw Gram G and
rn = 1/||e||; the explicit relu guards the masked diagonal against the bf16
rounding of rn.

Precision: embeddings as fp8_e4m3 (PE Gram in fp8; norms from the SAME fp8
values), rn/selector/broadcast matmuls in bf16 (single-pass PE), weights bf16.
Measured rel-err vs the fp32 reference ~1e-4.

Layout: [128, 192] -- partitions 0:48 anchors x k-block 0 (local k 0..191),
64:112 anchors x k-block 1; 48:64 and 112:128 are pad driven by junk lhsT
columns and masked by pm=0.  Local column order: [anchors | positives | rest],
so positives sit on diag of t2[0:48, 48:96].

Structure notes (from NTFF traces):
- one HWDGE ring sustains only ~150-200 GB/s; the embedding loads are split
  across the sync AND scalar rings (ACT table loads do NOT block the scalar
  sequencer's DMA issues).
- matmul rhs/out partition bases must be 0/32/64, so the three rn transposes
  land at partitions 0/32/64 of one PSUM tile; the RB broadcast panels then
  read partition-base-aligned rows.
- x1 = G * (-2 rn_a) runs on ACT (per-partition scale), freeing the DVE for
  the rn row copy that gates the RB panels.
- dpos^2 rides as column 192 of a [128,193] tile: one ACT sqrt covers the
  grid and the positive distances.
- count = sum(sign(lwpre - EPS)) on ACT in parallel with the DVE sum-reduce;
  host maps sign-sum -> count.
- the [1,2] result is written back by engine register stores (posted writes),
  skipping a ~1.3us DMA issue+latency.
"""

import os
import sys

for _p in ("/opt/trn_rl_repo", "/root/.axon_site/_ro/trn_rl_repo"):
    if _p not in sys.path:
        sys.path.append(_p)

if "jax" not in sys.modules and os.environ.get("JAX_PLATFORMS") in ("cpu",):
    del os.environ["JAX_PLATFORMS"]

import ml_dtypes
import numpy as np

import concourse.bass as bass
import concourse.tile as tile
from concourse import mybir
from concourse.bacc import Bacc
from concourse.bass_utils import run_bass_kernel_spmd

F32 = mybir.dt.float32
BF16 = mybir.dt.bfloat16
F8 = mybir.dt.float8e4
U32 = mybir.dt.uint32
ALU = mybir.AluOpType
ACT = mybir.ActivationFunctionType

B = 192
N = 2 * B
D = 512
NCORES = 8
S = N // NCORES          # 48 anchors per core
MARGIN = 0.1
EPS = 1e-8
DELTA = 1e-5
CELLS = 128 * 192 * NCORES
N_WARMUP = 5
REG_STORE_OUT = False    # engine stores to the XLA output buffer fault at runtime


def build_nc() -> bass.Bass:
    nc = Bacc()

    era = nc.dram_tensor("era", [128, 512], F8, kind="ExternalInput")
    erb = nc.dram_tensor("erb", [128, 1024], F8, kind="ExternalInput")
    et0 = nc.dram_tensor("et0", [128, 768], F8, kind="ExternalInput")
    et1 = nc.dram_tensor("et1", [128, 768], F8, kind="ExternalInput")
    pmw = nc.dram_tensor("pmw", [128, 192], BF16, kind="ExternalInput")
    cst = nc.dram_tensor("cst", [128, 448], BF16, kind="ExternalInput")
    out = nc.dram_tensor("out", [1, 2], F32, kind="ExternalOutput")

    with tile.TileContext(nc) as tc:
        with (
            tc.tile_pool(name="sb", bufs=1) as sb,
            tc.tile_pool(name="ps", bufs=1, space="PSUM") as ps,
        ):
            ET = sb.tile([128, 1536], F8, tag="ET")
            ER = sb.tile([128, 1536], F8, tag="ER")
            pm = sb.tile([128, 192], BF16, tag="pm")
            cs = sb.tile([128, 448], BF16, tag="cs")
            identB = cs[:, 0:128]          # identity
            sel2 = cs[0:48, 128:256]       # sel2[c,p]=1 iff p%64==c
            sel2m = cs[0:48, 256:384]      # -2 * sel2
            # ones rows live at partitions 0/32/64 (matmul base-match)

            # ---- DMAs: sync ring carries the norm chunks + et1; the scalar
            #      ring (not blocked by ACT table loads) carries et0 + pm;
            #      consts ride gpsimd SWDGE ----
            nc.sync.dma_start(out=ER[:, 0:512], in_=era[:, :])
            nc.sync.dma_start(out=ER[:, 512:1536], in_=erb[:, :])
            nc.sync.dma_start(out=ET[:, 768:1536], in_=et1[:, :])
            nc.scalar.dma_start(out=ET[:, 0:768], in_=et0[:, :])
            nc.scalar.dma_start(out=pm, in_=pmw[:, :])
            nc.gpsimd.dma_start(out=cs, in_=cst[:, :])

            # ---- DVE constants ----
            wtile = sb.tile([128, 256], F8, tag="wtile")
            nc.vector.memset(wtile, 1.0)
            onesc = sb.tile([128, 1], F32, tag="onesc")
            nc.vector.memset(onesc, 1.0)
            beps = sb.tile([128, 1], F32, tag="beps")
            nc.vector.memset(beps, -EPS)

            # ---- dummy sqrt pulls the ACT table early ----
            tdum = sb.tile([1, 1], F32, tag="tdum")
            nc.scalar.sqrt(tdum, onesc[0:1, 0:1])

            # ---- PE warm-up bridging the DMA phase ----
            wps = ps.tile([128, 256], F32, tag="wps")
            for _ in range(N_WARMUP):
                nc.tensor.matmul(wps, wtile[:, 0:128], wtile,
                                 start=True, stop=True)

            # ---- norms from the fp8 rows: DVE chunks 0,2; ACT chunk 1 ----
            ns_col = sb.tile([128, 3], F32, tag="ns_col")
            junk = sb.tile([128, 512], BF16, tag="junk")
            junk2 = sb.tile([128, 512], BF16, tag="junk2")
            nc.vector.scalar_tensor_tensor(
                junk, ER[:, 0:512], 1.0, ER[:, 0:512], op0=ALU.mult,
                op1=ALU.mult, accum_out=ns_col[:, 0:1])
            nc.scalar.activation(junk2, ER[:, 512:1024], ACT.Square,
                                 accum_out=ns_col[:, 1:2])
            nc.vector.scalar_tensor_tensor(
                junk, ER[:, 1024:1536], 1.0, ER[:, 1024:1536], op0=ALU.mult,
                op1=ALU.mult, accum_out=ns_col[:, 2:3])

            # ---- Gram in [128,192] layout: 2 blocks x 4 chunks, fp8 ----
            g_ps = ps.tile([128, 192], F32, tag="G")
            for c in range(4):
                lhsT = ET[:, 384 * c:384 * c + 64]
                nc.tensor.matmul(g_ps[0:64, :], lhsT,
                                 ET[:, 384 * c:384 * c + 192],
                                 start=(c == 0), stop=(c == 3))
                nc.tensor.matmul(g_ps[64:128, :], lhsT,
                                 ET[:, 384 * c + 192:384 * c + 384],
                                 start=(c == 0), stop=(c == 3))

            # ---- rn = 1/sqrt(ns) in bf16 (relu-guarded downstream) ----
            nrm = sb.tile([128, 3], F32, tag="nrm")
            nc.scalar.sqrt(nrm, ns_col)
            rn_col = sb.tile([128, 3], BF16, tag="rn_col")
            with nc.allow_low_precision("bf16 rn; relu-guarded downstream"):
                nc.vector.reciprocal(rn_col, nrm)

            # ---- rnA[p] = -2*rn[p%64] via selector matmul ----
            rnA_ps = ps.tile([128, 1], F32, tag="rnA")
            nc.tensor.matmul(rnA_ps, sel2m, rn_col[0:48, 0:1],
                             start=True, stop=True)
            rnAsb = sb.tile([128, 1], F32, tag="rnAsb")
            nc.scalar.copy(rnAsb, rnA_ps)

            # ---- rn to one partition-0 row [1,384] via 3 transposes ----
            rts_ps = ps.tile([1, 384], BF16, tag="rnT")
            for j in range(3):
                nc.tensor.transpose(rts_ps[0:1, 128 * j:128 * (j + 1)],
                                    rn_col[:, j:j + 1], identB)
            rrow = sb.tile([1, 384], BF16, tag="rrow")
            with nc.allow_low_precision("bf16 rn rows; relu-guarded"):
                nc.vector.tensor_copy(rrow, rts_ps)

            # ---- RB[p,f] = rn_loc[192*(p//64) + f] via 4 rank-1 panels ----
            ones1 = cs[0:1, 384:448]
            rb_ps = ps.tile([128, 192], F32, tag="RB")
            nc.tensor.matmul(rb_ps[0:64, 0:128], ones1, rrow[0:1, 0:128],
                             start=True, stop=True)
            nc.tensor.matmul(rb_ps[0:64, 128:192], ones1, rrow[0:1, 128:192],
                             start=True, stop=True)
            nc.tensor.matmul(rb_ps[64:128, 0:64], ones1, rrow[0:1, 192:256],
                             start=True, stop=True)
            nc.tensor.matmul(rb_ps[64:128, 64:192], ones1, rrow[0:1, 256:384],
                             start=True, stop=True)

            # ---- t2 = -2 * G * rn_a * rn_k (x1 on ACT frees the DVE) ----
            x1 = sb.tile([128, 192], F32, tag="x1")
            nc.scalar.activation(x1, g_ps, ACT.Copy, bias=0.0, scale=rnAsb)
            t2_ps = ps.tile([128, 192], F32, tag="t2")
            nc.vector.tensor_mul(t2_ps, x1, rb_ps)

            # ---- positive-pair t2 values -> bf16 -> duplicated [128,1] ----
            tpj = sb.tile([48, 48], F32, tag="tpj")
            t2pos = sb.tile([48, 1], BF16, tag="t2pos")
            with nc.allow_low_precision("bf16 dpos path; |err| ~3e-3 abs"):
                nc.vector.scalar_tensor_tensor(
                    tpj, t2_ps[0:48, 48:96], 1.0, identB[0:48, 0:48],
                    op0=ALU.mult, op1=ALU.mult, accum_out=t2pos)
            tp_ps = ps.tile([128, 1], F32, tag="tp")
            nc.tensor.matmul(tp_ps, sel2, t2pos, start=True, stop=True)

            # ---- d2 grid + dpos^2 col 192; one sqrt covers both ----
            d2r = sb.tile([128, 193], F32, tag="d2r")
            nc.vector.tensor_scalar(
                d2r[:, 0:192], t2_ps, 2.0 + DELTA, 0.0, op0=ALU.add,
                op1=ALU.max)
            nc.vector.tensor_scalar(
                d2r[:, 192:193], tp_ps, 2.0 + DELTA, 0.0, op0=ALU.add,
                op1=ALU.max)
            dms = sb.tile([128, 193], F32, tag="dms")
            nc.scalar.sqrt(dms, d2r)
            dposm = sb.tile([128, 1], F32, tag="dposm")
            nc.vector.tensor_scalar_add(dposm, dms[:, 192:193], MARGIN)

            # ---- weighted terms; sum on DVE, sign-count on ACT ----
            lwpre = sb.tile([128, 192], F32, tag="lwpre")
            nc.vector.scalar_tensor_tensor(
                lwpre, dms[:, 0:192], dposm, pm, op0=ALU.subtract,
                op1=ALU.mult)
            stacked = sb.tile([128, 2], F32, tag="stacked")
            lwj = sb.tile([128, 192], F32, tag="lwj")
            nc.vector.tensor_scalar(
                lwj, lwpre, 0.0, 0.0, op0=ALU.max, op1=ALU.add,
                accum_out=stacked[:, 0:1])
            sgj = sb.tile([128, 192], F32, tag="sgj")
            nc.scalar.activation(sgj, lwpre, ACT.Sign, bias=beps, scale=1.0,
                                 accum_out=stacked[:, 1:2])

            # ---- cross-partition reduce + writeback ----
            outp = ps.tile([1, 2], F32, tag="outp")
            nc.tensor.matmul(outp, onesc, stacked, start=True, stop=True)
            outs = sb.tile([1, 2], F32, tag="outs")
            nc.vector.tensor_copy(outs, outp)
            if REG_STORE_OUT:
                outsi = outs.bitcast(U32)
                outi = out.bitcast(U32)
                with nc.vector.register("ro0") as r0:
                    nc.vector.reg_load(r0, outsi[0:1, 0:1])
                    nc.vector.store(outi[0:1, 0:1], r0)
                with nc.vector.register("ro1") as r1:
                    nc.vector.reg_load(r1, outsi[0:1, 1:2])
                    nc.vector.store(outi[0:1, 1:2], r1)
            else:
                nc.sync.dma_start(out=out[:, :], in_=outs)

    nc.finalize()
    return nc


_NC_CACHE: dict = {}


def _get_nc() -> bass.Bass:
    if "nc" not in _NC_CACHE:
        _NC_CACHE["nc"] = build_nc()
    return _NC_CACHE["nc"]


def _make_consts() -> np.ndarray:
    cst = np.zeros((128, 448), dtype=np.float32)
    cst[:, 0:128] = np.eye(128)
    sel = np.zeros((48, 128), dtype=np.float32)
    p = np.arange(128)
    for c in range(48):
        sel[c, p % 64 == c] = 1.0
    cst[0:48, 128:256] = sel
    cst[0:48, 256:384] = -2.0 * sel
    cst[0, 384:448] = 1.0
    cst[32, 384:448] = 1.0
    cst[64, 384:448] = 1.0
    return cst.astype(ml_dtypes.bfloat16)


_CST = _make_consts()


def make_in_maps(output1, output2, weight):
    o1 = np.asarray(output1, dtype=np.float32)
    o2 = np.asarray(output2, dtype=np.float32)
    w = np.asarray(weight, dtype=np.float32)

    emb = np.concatenate([o1, o2], axis=0)
    w2 = np.tile(w, (2, 2))
    f8 = ml_dtypes.float8_e4m3
    a48 = np.arange(S)

    in_maps = []
    for c in range(NCORES):
        anchors = np.arange(c * S, c * S + S)
        pos = (anchors + B) % N
        used = np.zeros(N, dtype=bool)
        used[anchors] = True
        used[pos] = True
        loc = np.concatenate([anchors, pos, np.nonzero(~used)[0]])

        emb_loc = np.ascontiguousarray(emb[loc])
        embt = emb_loc.T
        ET = np.concatenate([embt[128 * k:128 * (k + 1), :] for k in range(4)],
                            axis=1).astype(f8)
        ER = np.concatenate([emb_loc[128 * t:128 * (t + 1), :] for t in range(3)],
                            axis=1).astype(f8)

        pmn = np.zeros((128, 192), dtype=np.float32)
        pmn[0:48, :] = -w2[anchors[:, None], loc[None, 0:192]]
        pmn[64:112, :] = -w2[anchors[:, None], loc[None, 192:384]]
        pmn[a48, a48] = 0.0          # k == i
        pmn[a48, S + a48] = 0.0      # k == p(i)

        in_maps.append({
            "era": np.ascontiguousarray(ER[:, 0:512]),
            "erb": np.ascontiguousarray(ER[:, 512:1536]),
            "et0": np.ascontiguousarray(ET[:, 0:768]),
            "et1": np.ascontiguousarray(ET[:, 768:1536]),
            "pmw": pmn.astype(ml_dtypes.bfloat16),
            "cst": _CST,
        })
    return in_maps


def reduce_outputs(results):
    parts = np.stack([np.asarray(r["out"][0], dtype=np.float64)
                      for r in results])
    total = parts.sum(axis=0)
    count = (total[1] + CELLS) / 2.0
    return np.asarray(
        np.float32(total[0]) / (np.float32(count) + np.float32(EPS)),
        dtype=np.float32)


def kernel(output1, output2, weight):
    in_maps = make_in_maps(output1, output2, weight)
    res = run_bass_kernel_spmd(_get_nc(), in_maps, core_ids=list(range(NCORES)))
    return reduce_outputs(res.results)
